# revision 53
# baseline (speedup 1.0000x reference)
"""GNN message-passing ConvNet layer on 8 TRN2 NeuronCores (Bass/Tile).

Computes, for x [B=4, N=4096, D=128], adj_mat [B, N, N] (0/1 floats),
U [D, D]:
    mask = (adj_mat > 0)
    deg[b, i] = sum_j adj_mat[b, j, i]
    agg[b, i, :] = sum_j mask[b, j, i] * x[b, j, :]
    out = relu((agg @ U) / deg[..., None])

Sharding: split the destination node axis i. Core c handles batch c//2
and destination half c%2: it reads its own column slice
adj[b, :, i0:i0+2048] plus all of x[b]; no collectives.

fp8 design: adj is 0/1 so it is EXACT in fp8e4m3 -> adjacency HBM
traffic drops 4x vs fp32 (8.4 MiB/core). x is split hi/lo into two
fp8e4m3 planes (x ~= x_hi + x_lo) so the aggregation keeps ~bf16
precision. All aggregation matmuls run in fp8 DoubleRow mode (K=256
per matmul via j-pairs [j, j+128] in the Ko=2 weight slots).

deg comes FREE from the hi plane: x_hi[:, 0] is forced to exactly 1.0
(the real x[:,0] moves to x_lo[:, 0] = fp8(x0 - 1)), and the hi and lo
planes accumulate into SEPARATE PSUM banks. Row 0 of the hi PSUM is
then exactly deg (a sum of adj 1.0s in fp32), so no dedicated deg pass
is needed - that saves 1/3 of all PE work. The round tail computes
recip(deg) on DVE, broadcasts it, sums hi+lo and scales, then one bf16
U-matmul per half-round (U stationary, out^T layout; host transposes).

adj is pre-packed on host to [round][p][t2][k][i] fp8 (j = t2*256 +
k*128 + p) so every DMA line is contiguous per partition. The sync and
scalar queues (the two hardware DGE lanes) alternate chunks so DMA
posting latencies overlap; PE warm-up matmuls on memset scratch cover
the initial DMA latency and un-throttle the HAM clock gate.
"""

import os
import sys

for _p in ("/opt/trn_rl_repo",):
    if _p not in sys.path and os.path.isdir(_p):
        sys.path.insert(0, _p)

from contextlib import ExitStack

import numpy as np

B, N, D = 4, 4096, 128
P = 128
N_CORES = 8

_PROG = None


def _build_program(n=N, i_core=N // 2, d=D, w=512):
    from concourse import mybir, tile, bacc

    f32 = mybir.dt.float32
    bf16 = mybir.dt.bfloat16
    fp8 = mybir.dt.float8e4
    DR = mybir.MatmulPerfMode.DoubleRow
    n_t2 = n // (2 * P)  # 16 j-blocks of 256 rows
    n_rounds = i_core // w

    nc = bacc.Bacc(
        "TRN2",
        target_bir_lowering=False,
        debug=False,
        enable_asserts=True,
        num_devices=N_CORES,
    )
    adj_d = nc.dram_tensor(
        "adj_q", [n_rounds, P, n_t2, 2, w], fp8, kind="ExternalInput"
    )
    xhi_d = nc.dram_tensor("x_hi", [P, n_t2, 2, d], fp8, kind="ExternalInput")
    xlo_d = nc.dram_tensor("x_lo", [P, n_t2, 2, d], fp8, kind="ExternalInput")
    u_d = nc.dram_tensor("U", [d, d], bf16, kind="ExternalInput")
    # output out^T in [e][round][i]; host transposes back to [i, e].
    out_d = nc.dram_tensor("out_sp", [P, n_rounds, w], bf16, kind="ExternalOutput")

    with tile.TileContext(nc, trace_sim=False) as tc, ExitStack() as ctx:
        const_pool = ctx.enter_context(tc.tile_pool(name="const", bufs=1))
        adj_pool = ctx.enter_context(tc.tile_pool(name="adj", bufs=10))
        scale_pool = ctx.enter_context(tc.tile_pool(name="scale", bufs=2))
        out_pool = ctx.enter_context(tc.tile_pool(name="out", bufs=2))
        small_pool = ctx.enter_context(tc.tile_pool(name="small", bufs=2))
        ps_agg = ctx.enter_context(tc.tile_pool(name="ps_agg", bufs=2, space="PSUM"))
        ps_out = ctx.enter_context(tc.tile_pool(name="ps_out", bufs=2, space="PSUM"))

        warm_w = const_pool.tile([P, 2, 16], fp8)
        nc.vector.memset(warm_w[:], 1.0)
        warm_sb = const_pool.tile([P, 2, 256], fp8)
        nc.vector.memset(warm_sb[:], 0.0)
        for _ in range(11):
            warm_ps = ps_out.tile([P, w // 2], f32, tag="o20")
            nc.tensor.matmul(
                warm_ps[0:16, 0:256],
                warm_w[:],
                warm_sb[:],
                start=True,
                stop=True,
                perf_mode=DR,
            )
        # Each hardware DGE lane (sync=SP, scalar=Activation) is a serial
        # posting pipe with ~3us per-DMA latency; gpsimd's software DGE is
        # far slower - keep it off the critical path. First chunk leads
        # the sync lane, x_hi leads scalar; x_lo is scalar's 2nd (round 0
        # defers its lo matmuls to a late sweep so that is in time).
        xhi_sb = const_pool.tile([P, n_t2, 2, d], fp8)
        nc.scalar.dma_start(xhi_sb[:], xhi_d[:])
        pre_adj = []
        for eng, t0 in ((nc.sync, 0), (nc.sync, 1)):
            t_adj = adj_pool.tile([P, 1, 2, w], fp8, tag="adj")
            eng.dma_start(t_adj[:], adj_d[0, :, t0 : t0 + 1, :, :])
            pre_adj.append(t_adj)
        # x_lo (in two half-tiles) and U are allocated here but their DMAs
        # are issued lazily inside the round loop, interleaved with round
        # 0's scalar-lane chunks: the lane posts serially, and round 0
        # only needs x_lo at its end-of-round lo sweep (the first sweep
        # half can start as soon as xlo_a lands) and U at its tail.
        xlo_a = const_pool.tile([P, n_t2 // 2, 2, d], fp8)
        xlo_b = const_pool.tile([P, n_t2 // 2, 2, d], fp8)
        u_sb = const_pool.tile([P, d], bf16)

        def xlo_slice(tt):
            if tt < n_t2 // 2:
                return xlo_a[:, tt, :, :]
            return xlo_b[:, tt - n_t2 // 2, :, :]

        def emit_tail(q, hi_ps, lo_ps, split=2, fuse_lo=False):
            """Round tail: recip(deg) from hi row 0, broadcast, hi+lo sum
            and scale, one U-matmul per half, ReLU, store. Emitted one
            round late so the PE FIFO never stalls on it; split halves
            pipeline across DVE/GpSimd/PE. With fuse_lo the hi and lo
            planes are scaled separately and SUMMED BY the U-matmul's
            PSUM accumulation - shorter DVE chain, one extra (cheap)
            matmul per half; used for the last round where the PE is
            otherwise idle-waiting on the DVE."""
            ws = w // split
            for h in range(split):
                sl = slice(h * ws, (h + 1) * ws)
                recip = small_pool.tile([1, ws], f32, tag=f"recip{h}")
                nc.vector.reciprocal_approx_fast(recip[:], hi_ps[0:1, sl])
                rb = scale_pool.tile([P, ws], f32, tag=f"rb{h}")
                nc.gpsimd.partition_broadcast(rb[:], recip[:])
                o_ps = ps_out.tile([P, ws], f32, tag=f"o2{h}")
                if fuse_lo:
                    aggs_h = scale_pool.tile([P, ws], bf16, tag=f"aggs{h}")
                    nc.vector.tensor_mul(aggs_h[:], hi_ps[:, sl], rb[:])
                    aggs_l = scale_pool.tile([P, ws], bf16, tag=f"aggl{h}")
                    nc.vector.tensor_mul(aggs_l[:], lo_ps[:, sl], rb[:])
                    nc.tensor.matmul(
                        o_ps[:], u_sb[:], aggs_h[:], start=True, stop=False
                    )
                    nc.tensor.matmul(
                        o_ps[:], u_sb[:], aggs_l[:], start=False, stop=True
                    )
                else:
                    lo_sb = scale_pool.tile([P, ws], f32, tag=f"lo{h}")
                    nc.vector.tensor_copy(lo_sb[:], lo_ps[:, sl])
                    ssum = scale_pool.tile([P, ws], f32, tag=f"sum{h}")
                    nc.vector.tensor_add(ssum[:], hi_ps[:, sl], lo_sb[:])
                    aggs = scale_pool.tile([P, ws], bf16, tag=f"aggs{h}")
                    nc.vector.tensor_mul(aggs[:], ssum[:], rb[:])
                    nc.tensor.matmul(
                        o_ps[:], u_sb[:], aggs[:], start=True, stop=True
                    )
                out_sb = out_pool.tile([P, ws], bf16, tag=f"osb{h}")
                nc.vector.tensor_relu(out_sb[:], o_ps[:])
                nc.scalar.dma_start(out_d[:, q, sl], out_sb[:])

        def chunks_for(q):
            if q == 0:
                return [1, 1, 2, 4, 4, 4]
            return [4, 4, 4, 4]

        dma_count = 0
        pending = None
        for q in range(n_rounds):
            hi_ps = ps_agg.tile([P, w], f32, tag="hi")
            lo_ps = ps_agg.tile([P, w], f32, tag="lo")
            chunk_t2 = chunks_for(q)
            # Rounds 0 and last defer all lo matmuls to an end-of-round
            # sweep. Round 0: x_lo's DMA is off the critical path and the
            # adj stream catches up by sweep time. Last round: the hi
            # plane (and thus deg) completes ~3.5us before the lo plane,
            # so the tail's recip/broadcast overlap the sweep instead of
            # trailing the final matmul.
            defer = q == 0 or q == n_rounds - 1
            lo_lag = len(chunk_t2) + 1 if defer else 0
            lo_queue = []
            lo_emitted = 0

            def emit_lo(n_items):
                nonlocal lo_emitted
                for tt, a_sb, t in lo_queue[lo_emitted : lo_emitted + n_items]:
                    nc.tensor.matmul(
                        lo_ps[:],
                        xlo_slice(tt),
                        a_sb[:, t, :, :],
                        start=(lo_emitted == 0),
                        stop=(lo_emitted == n_t2 - 1),
                        perf_mode=DR,
                    )
                    lo_emitted += 1

            t0 = 0
            for c, ct in enumerate(chunk_t2):
                if q == 0 and c < len(pre_adj):
                    adj_sb = pre_adj[c]
                else:
                    adj_sb = adj_pool.tile([P, ct, 2, w], fp8, tag="adj")
                    eng = nc.sync if (dma_count % 2 == 0) else nc.scalar
                    dma_count += 1
                    eng.dma_start(adj_sb[:], adj_d[q, :, t0 : t0 + ct, :, :])
                first, last = c == 0, c == len(chunk_t2) - 1
                for t in range(ct):
                    nc.tensor.matmul(
                        hi_ps[:],
                        xhi_sb[:, t0 + t, :, :],
                        adj_sb[:, t, :, :],
                        start=(first and t == 0),
                        stop=(last and t == ct - 1),
                        perf_mode=DR,
                    )
                    lo_queue.append((t0 + t, adj_sb, t))
                    if lo_lag == 0:
                        emit_lo(1)
                t0 += ct
                if q == 0 and c == 3:
                    # scalar lane, right after its first round-0 chunk:
                    # first x_lo half (needed by the sweep's first wave).
                    nc.scalar.dma_start(xlo_a[:], xlo_d[:, 0 : n_t2 // 2])
                if q == 0 and c == len(chunk_t2) - 1:
                    # second x_lo half, then U (needed at the round-0
                    # tail, which is emitted during round 1).
                    nc.scalar.dma_start(xlo_b[:], xlo_d[:, n_t2 // 2 :])
                    nc.scalar.dma_start(u_sb[:], u_d[:])
            emit_lo(len(lo_queue) - lo_emitted)
            if pending is not None:
                emit_tail(*pending)
            pending = (q, hi_ps, lo_ps)
        emit_tail(*pending)

    nc.compile()
    return nc


def _get_program():
    global _PROG
    if _PROG is None:
        _PROG = _build_program()
    return _PROG


def _adj_to_fp8_exact(slab_f32):
    """adj values are 0/1: map directly to the fp8e4m3 bit patterns."""
    import ml_dtypes

    u = np.where(slab_f32 != 0, np.uint8(0x38), np.uint8(0)).astype(np.uint8)
    return u.view(ml_dtypes.float8_e4m3)


def _shard_inputs(x, adj_mat, U):
    import ml_dtypes

    e4 = ml_dtypes.float8_e4m3
    i_core = N // 2
    w = 512
    n_rounds = i_core // w
    n_t2 = N // (2 * P)
    u_bf = np.ascontiguousarray(U.astype(ml_dtypes.bfloat16))
    in_maps = []
    xcache = {}
    for c in range(N_CORES):
        b, half = c // 2, c % 2
        i0 = half * i_core
        # adj slab [4096 j, 2048 i] -> [q][p][t2][k][ii] with
        # j = t2*256 + k*128 + p, i = q*512 + ii.
        slab = adj_mat[b, :, i0 : i0 + i_core]
        packed = _adj_to_fp8_exact(slab).reshape(n_t2, 2, P, n_rounds, w)
        packed = np.ascontiguousarray(packed.transpose(3, 2, 0, 1, 4))
        if b not in xcache:
            xb = x[b]
            x_hi = xb.astype(e4)
            x_lo = (xb - x_hi.astype(np.float32)).astype(e4)
            # deg-from-hi trick: hi column 0 is exactly 1.0; the real
            # x[:, 0] moves to the lo plane as fp8(x0 - 1).
            x_hi[:, 0] = e4(1.0)
            x_lo[:, 0] = (xb[:, 0] - 1.0).astype(e4)

            # [j, d] -> [p, t2, k, d] with j = t2*256 + k*128 + p
            def pack_x(a):
                return np.ascontiguousarray(
                    a.reshape(n_t2, 2, P, D).transpose(2, 0, 1, 3)
                )

            xcache[b] = (pack_x(x_hi), pack_x(x_lo))
        xhi_p, xlo_p = xcache[b]
        in_maps.append(
            {
                "adj_q": packed,
                "x_hi": xhi_p,
                "x_lo": xlo_p,
                "U": u_bf,
            }
        )
    return in_maps


def _run(x, adj_mat, U, trace=False):
    from concourse.bass_utils import run_bass_kernel_spmd

    nc = _get_program()
    in_maps = _shard_inputs(x, adj_mat, U)
    res = run_bass_kernel_spmd(
        nc, in_maps, core_ids=list(range(N_CORES)), trace=trace
    )
    i_core = N // 2
    out = np.empty((B, N, D), dtype=np.float32)
    for c in range(N_CORES):
        b, half = c // 2, c % 2
        i0 = half * i_core
        osp = res.results[c]["out_sp"].astype(np.float32)  # [e, q, i]
        out[b, i0 : i0 + i_core, :] = osp.transpose(1, 2, 0).reshape(i_core, D)
    return out, res


def kernel(x, adj_mat, U):
    out, _ = _run(
        np.asarray(x, dtype=np.float32),
        np.asarray(adj_mat, dtype=np.float32),
        np.asarray(U, dtype=np.float32),
    )
    return out
